# revision 56
# baseline (speedup 1.0000x reference)
"""Trainium2 Bass kernel for nn_GATNet_IMG (dense 2-layer GAT, N=4096).

Sharding: 1D row-parallel over the node dim across 8 NeuronCores.
Each core computes Wh for its 512 rows (all 4 heads), AllGathers Wh
per head (overlapped with the next head's matmuls), then computes its
[512, 4096] attention block per head with a fused masked softmax (no
NxN matrix ever hits HBM), aggregates h^T = Wh^T @ u on TensorE, and
repeats the same pattern for the output attention layer.

Key design points:
  - both attention-aggregate matmul operands are BF16 (mixed
    fp16/bf16 streams the PE at half rate); fp8 DoubleRow for phase 1
    is implemented behind PH1_FP8 but off — it computes exactly
    (verified vs host) yet the e4m3 input quantization alone costs
    ~5e-2 final rel err on the graded inputs, over the 2e-2 gate
  - phase-3 k-tiles run incrementally per head inside the phase-2
    PSUM-drain windows; the ag2 gather is split into two collectives
    so the first half of phase 4 overlaps the second collective
  - exp factorization: exp(leaky(f1+f2)+c) = max(E1[i]G2[j],
    E1a[i]G2a[j]) with E1=exp(f1), G2=exp(f2+c) etc., so the per-tile
    work is one ACT per-partition-scale multiply plus one fused DVE
    scalar_tensor_tensor (mult+max) instead of two full exps
  - f1/f2 logit halves precomputed on host in fp32 (tiny GEMM), so
    logit precision is independent of the big-GEMM compute dtype
  - unnormalized attention + broadcast row-sum via all-ones matmul;
    normalization is a fast-approx reciprocal postscale
  - elu(x) == max(exp(min(x, 0)) - 1, x)           (exact)
  - ln(2^-30) folded into the exp bias keeps row sums in range
  - dual DMA queues: bulk streams on SP, latency-critical prefetches
    (next-phase Wh blocks, adj) on the Activation HWDGE queue
  - host-side sharding pre-transposes x/adj and interleaves heavy
    streams into [128, ktiles, free] partition-major layouts
"""

import math
from contextlib import ExitStack

import numpy as np

import concourse.bass as bass
import concourse.mybir as mybir
import concourse.tile as tile
from concourse import bass_utils
from concourse.masks import make_identity

F32 = mybir.dt.float32
F16 = mybir.dt.float16
BF16 = mybir.dt.bfloat16
F8 = mybir.dt.float8e4
AF = mybir.ActivationFunctionType
OP = mybir.AluOpType
DR = mybir.MatmulPerfMode.DoubleRow

N = 4096
NFEAT = 4096
NHID = 1024
NHEADS = 4
BIT = 64
NC = 8
R = N // NC          # 512 attention rows per core
KT = NFEAT // 128    # 32 k tiles
JT = N // 128        # 32 node-column tiles
IT = R // 128        # 4 row tiles per core
ALPHA = 0.2
BIAS_LN = -30.0 * math.log(2.0)   # ln(2^-30) folded into exp bias (layer 1)
W2 = BIT + 2         # packed ag2 row: 64 bf16 Wh2 + 1.0 + bf16 g2
PH1_FP8 = False       # phase-1 GEMM in fp8 e4m3 DoubleRow vs fp16
XS = 8.0             # fp8 phase-1 input scales (x*XS, W*WS; both <= 240,
WS = 128.0           # and product scale small enough that Wh*XS*WS stays
                     # far from the fp16 range in case the PE's DoubleRow
                     # path carries reduced-precision partials)
INV_SCALE = 1.0 / (XS * WS) if PH1_FP8 else 1.0


def _split_excess_waits(nc, max_waits=1):
    """walrus codegen rejects >max_waits sync-wait commands per instruction;
    push excess waits onto preceding same-engine NoOps."""
    n_fixed = 0
    for f in nc.m.functions:
        for b in f.blocks:
            new_insts = []
            changed = False
            for inst in b.instructions:
                si = getattr(inst, "sync_info", None)
                if si is not None and si.on_wait and len(si.on_wait) > max_waits:
                    waits = list(si.on_wait)
                    excess, keep = waits[:-max_waits], waits[-max_waits:]
                    for ci in range(0, len(excess), max_waits):
                        nop = mybir.InstNoOp(
                            name=f"{inst.name}-ws{ci}",
                            sync_info=mybir.SyncInfo(
                                on_wait=excess[ci:ci + max_waits], on_update=[]
                            ),
                            bass_nofuse=True,
                            engine=inst.engine,
                        )
                        new_insts.append(nop)
                    inst.sync_info = mybir.SyncInfo(
                        on_wait=keep, on_update=list(si.on_update or [])
                    )
                    n_fixed += 1
                    changed = True
                new_insts.append(inst)
            if changed:
                insts = b.instructions
                try:
                    b.instructions = new_insts
                except Exception:
                    while len(insts):
                        insts.pop()
                    for i in new_insts:
                        insts.append(i)
    return n_fixed


def build_program():
    nc = bass.Bass("TRN2", target_bir_lowering=False, debug=False, num_devices=NC)

    PH1_DT = F8 if PH1_FP8 else F16
    # host-interleaved inputs: [128 partitions, ktiles, free]
    x_d = nc.dram_tensor("x_sh", [128, KT, R], PH1_DT, kind="ExternalInput").ap()
    W_d = nc.dram_tensor("W_sh", [NHEADS, 128, KT, NHID], PH1_DT,
                         kind="ExternalInput").ap()
    adj_d = nc.dram_tensor("adj_sh", [128, JT, R], BF16, kind="ExternalInput").ap()
    wo_d = nc.dram_tensor("Wo_sh", [128, KT, BIT], F16, kind="ExternalInput").ap()
    E1_d = nc.dram_tensor("E1_sh", [128, NHEADS, R], BF16,
                          kind="ExternalInput").ap()
    E1a_d = nc.dram_tensor("E1a_sh", [128, NHEADS, R], BF16,
                           kind="ExternalInput").ap()
    G2_d = nc.dram_tensor("G2_sh", [128, NHEADS, IT, NC], F32,
                          kind="ExternalInput").ap()
    G2a_d = nc.dram_tensor("G2a_sh", [128, NHEADS, IT, NC], F32,
                           kind="ExternalInput").ap()
    a1o_d = nc.dram_tensor("a1_out", [BIT], F32, kind="ExternalInput").ap()
    a2o_d = nc.dram_tensor("a2_out", [BIT], F32, kind="ExternalInput").ap()
    out_d = nc.dram_tensor("out_rows", [R, BIT], F32, kind="ExternalOutput").ap()

    # collective bounce buffers; Wh per head so each head's allgather overlaps
    # the next head's phase-1 compute.
    ag1_in = [nc.dram_tensor(f"ag1_in{h}", [128, IT * NHID], BF16).ap()
              for h in range(NHEADS)]
    ag1_out = [nc.dram_tensor(f"ag1_out{h}", [NC * 128, IT * NHID], BF16,
                              addr_space="Shared").ap() for h in range(NHEADS)]
    # ag2 in two halves (i-blocks 0-1 / 2-3) so the first collective and
    # the first half of phase 4 overlap the second collective
    ag2_in2 = [nc.dram_tensor(f"ag2_in{half}", [128, 2 * W2], BF16).ap()
               for half in range(2)]
    ag2_out2 = [nc.dram_tensor(f"ag2_out{half}", [NC * 128, 2 * W2], BF16,
                               addr_space="Shared").ap() for half in range(2)]

    rg = [list(range(NC))]

    with tile.TileContext(nc) as tc, ExitStack() as ctx:
        cp = ctx.enter_context(tc.tile_pool(name="const", bufs=1))
        ident = cp.tile([128, 128], F32)
        make_identity(nc, ident)
        ones128 = cp.tile([128, 128], F32)
        nc.vector.memset(ones128, 1.0)
        a1o_col = cp.tile([BIT, 1], F32)
        a2o_b = cp.tile([128, BIT], F32)
        ones_row = cp.tile([1, 128], F32)
        nc.vector.memset(ones_row, 1.0)
        # logit tables: host-precomputed exps of the f1/f2 halves
        G2 = cp.tile([128, NHEADS, IT, NC], F32)
        G2a = cp.tile([128, NHEADS, IT, NC], F32)
        E1b = cp.tile([128, NHEADS, R], BF16)
        E1ab = cp.tile([128, NHEADS, R], BF16)
        # adjacency mask, resident for both attention layers
        adjT = cp.tile([128, JT, R], BF16)
        wob = cp.tile([128, KT, BIT], F16)
        # phase-2 head-0 Wh prefetch (filled mid-phase-1 via the ACT queue)
        whtA = [cp.tile([128, IT, NHID], BF16, name=f"whtA{c}") for c in range(2)]
        # phase-4 Wh2 blocks, all 8 chunks resident (4.2 KiB/partition)
        w2all = cp.tile([128, NC, IT, W2], BF16)

        # =============== phase 1: Wh = x @ W[h] ===============
        with tc.tile_pool(name="p0", bufs=1) as p0, \
             tc.tile_pool(name="p1s", bufs=4) as p1s, \
             tc.tile_pool(name="p1ps", bufs=1, space="PSUM") as p1ps, \
             tc.tile_pool(name="p1d", bufs=3) as p1d:
            xp1 = p0.tile([128, KT, R], PH1_DT)
            for q in range(8):
                nc.sync.dma_start(xp1[:, q * 4:(q + 1) * 4, :],
                                  x_d[:, q * 4:(q + 1) * 4, :])
            for h in range(NHEADS):
                ps = [[p1ps.tile([128, 512], F32, name=f"ps_{h}_{i}_{oh}",
                                 tag=f"ps{i}{oh}") for oh in range(2)]
                      for i in range(IT)]
                for kb in range(4):
                    wres = p1s.tile([128, 8, NHID], PH1_DT, tag="wres")
                    if h == 0:
                        if kb == 0:
                            nc.scalar.dma_start(
                                wres[:, :2, :], W_d[0, :, 0:2, :])
                            nc.scalar.dma_start(
                                wres[:, 2:, :], W_d[0, :, 2:8, :])
                            # tiny phase-3 vectors behind the critical W chunk
                            nc.scalar.dma_start(
                                a1o_col, a1o_d.rearrange("(b one) -> b one",
                                                         one=1))
                            nc.scalar.dma_start(
                                a2o_b, a2o_d.rearrange(
                                    "(one b) -> one b",
                                    one=1).to_broadcast([128, BIT]))
                        else:
                            nc.scalar.dma_start(
                                wres, W_d[0, :, kb * 8:(kb + 1) * 8, :])
                        if kb == 3:
                            # logit tables + adj behind all four W chunks
                            nc.scalar.dma_start(E1b, E1_d)
                            nc.scalar.dma_start(E1ab, E1a_d)
                            nc.scalar.dma_start(G2, G2_d)
                            nc.scalar.dma_start(G2a, G2a_d)
                            nc.scalar.dma_start(adjT, adj_d)
                    else:
                        nc.sync.dma_start(wres, W_d[h, :, kb * 8:(kb + 1) * 8, :])
                    if PH1_FP8:
                        for kk2 in range(4):
                            k2 = kb * 4 + kk2
                            for i in range(IT):
                                for oh in range(2):
                                    nc.tensor.matmul(
                                        ps[i][oh],
                                        lhsT=xp1[:, 2 * k2:2 * k2 + 2,
                                                 i * 128:(i + 1) * 128],
                                        rhs=wres[:, 2 * kk2:2 * kk2 + 2,
                                                 oh * 512:(oh + 1) * 512],
                                        start=(k2 == 0),
                                        stop=(k2 == KT // 2 - 1),
                                        perf_mode=DR,
                                    )
                    else:
                        for kk in range(8):
                            k = kb * 8 + kk
                            for i in range(IT):
                                for oh in range(2):
                                    nc.tensor.matmul(
                                        ps[i][oh],
                                        lhsT=xp1[:, k, i * 128:(i + 1) * 128],
                                        rhs=wres[:, kk,
                                                 oh * 512:(oh + 1) * 512],
                                        start=(k == 0), stop=(k == KT - 1),
                                    )
                if h == 0:
                    nc.scalar.dma_start(wob, wo_d)
                for i in range(IT):
                    wh_sb = p1d.tile([128, NHID], BF16, tag="wh_sb")
                    nc.vector.tensor_scalar_mul(wh_sb[:, :512], ps[i][0],
                                                INV_SCALE)
                    nc.scalar.mul(wh_sb[:, 512:], ps[i][1], INV_SCALE)
                    # scalar queue: the sync queue stays a pure x/W stream,
                    # so the next head's W chunks are never stuck behind
                    # these drain-gated writes
                    nc.scalar.dma_start(
                        ag1_in[h][:, i * NHID:(i + 1) * NHID], wh_sb)
                # allgather this head's Wh while later heads compute
                nc.gpsimd.collective_compute(
                    "AllGather", OP.bypass, ins=[ag1_in[h].opt()],
                    outs=[ag1_out[h].opt()], replica_groups=rg)
                if h == 1:
                    # prefetch head-0's first attention Wh blocks on the Pool
                    # queue: Pool is idle all of phase 1, so its blocking
                    # wait on ag1[0] completion head-of-line-blocks nothing
                    for c in range(2):
                        nc.gpsimd.dma_start(
                            whtA[c], ag1_out[0][c * 128:(c + 1) * 128, :].rearrange(
                                "p (i o) -> p i o", i=IT))

        # =============== phase 2: attention + aggregate, per head ===============
        p2c = ctx.enter_context(tc.tile_pool(name="p2c", bufs=1))
        xcatT = p2c.tile([128, KT, R], F16)

        pps = ctx.enter_context(tc.tile_pool(name="pps", bufs=1, space="PSUM"))
        p2s = ctx.enter_context(tc.tile_pool(name="p2s", bufs=2))
        p2w = ctx.enter_context(tc.tile_pool(name="p2w", bufs=2))
        p2p = ctx.enter_context(tc.tile_pool(name="p2p", bufs=4))
        # phase-3 running sum, fed incrementally as each head's xcat lands
        wh2sb = p2c.tile([BIT, R], F32)

        for h in range(NHEADS):
            rsA = p2s.tile([128, R], F32, tag="rsA")
            nc.gpsimd.memset(rsA, 0.0)

            hps = [pps.tile([128, R], F32, name=f"hps{h}_{os}", tag=f"h{os}")
                   for os in range(8)]
            for c in range(NC):
                if h == 0 and c < 2:
                    wht4 = whtA[c]
                else:
                    wht4 = p2w.tile([128, IT, NHID], BF16, tag="wht", bufs=3)
                    nc.sync.dma_start(
                        wht4, ag1_out[h][c * 128:(c + 1) * 128, :].rearrange(
                            "p (i o) -> p i o", i=IT))
                for i in range(IT):
                    if h > 0 and i == 2:
                        # previous head's elu tail, mid-chunk so the DVE ops
                        # don't delay this chunk's first u tiles
                        _elu_tail(c)
                    jt = c * IT + i
                    e2 = p2p.tile([128, R], BF16, tag="e2")
                    nc.scalar.mul(e2, E1ab[:, h, :], G2a[:, h, i, c:c + 1])
                    m = p2p.tile([128, R], BF16, tag="m")
                    if jt % 2 == 0:
                        nc.vector.scalar_tensor_tensor(
                            out=m, in0=E1b[:, h, :], scalar=G2[:, h, i, c:c + 1],
                            in1=e2, op0=OP.mult, op1=OP.max)
                    else:
                        e1 = p2p.tile([128, R], BF16, tag="e1")
                        nc.scalar.mul(e1, E1b[:, h, :], G2[:, h, i, c:c + 1])
                        nc.vector.tensor_tensor(m, e1, e2, OP.max)
                    u = p2p.tile([128, R], BF16, tag="u")
                    nc.vector.tensor_tensor(u, m, adjT[:, jt, :], OP.mult)
                    nc.gpsimd.tensor_tensor(rsA, rsA, u, OP.add)
                    for os in range(8):
                        nc.tensor.matmul(
                            hps[os], lhsT=wht4[:, i, os * 128:(os + 1) * 128],
                            rhs=u, start=(jt == 0), stop=(jt == JT - 1))

            # eager PSUM drain (banks freed for the next head asap), then the
            # broadcast row-sum + approx-reciprocal normalization chain
            hsb = [p2s.tile([128, R], F32, name=f"hsb{h}_{os}", tag=f"hsb{os}",
                            bufs=1)
                   for os in range(8)]
            # DVE drains in stop order (os 0 stops first in jt=31's os-loop);
            # ACT drains os 7 first so the rowsum matmul (which WARs on
            # hsb[7]) unblocks as early as possible
            for os in (0, 2, 4, 6):
                nc.vector.tensor_copy(hsb[os], hps[os])
            for os in (7, 1, 3, 5):
                nc.scalar.copy(hsb[os], hps[os])
            if h in (1, 2):
                # heads 0/1's phase-3 k-tiles fill the drain windows: PSUM
                # h7 frees first and the PE would otherwise idle on drains.
                # Heads 2-3 stay in the tail so the PE has work to chew
                # while head 3's elu chain runs.
                hp = h - 1
                ph3 = pps.tile([BIT, R], F32, name=f"ph3_{hp}", tag="h7")
                for kk in range(8):
                    nc.tensor.matmul(
                        ph3, lhsT=wob[:, hp * 8 + kk, :],
                        rhs=xcatT[:, hp * 8 + kk, :],
                        start=(kk == 0), stop=(kk == 7))
                if hp == 0:
                    nc.scalar.copy(wh2sb, ph3)
                else:
                    nc.vector.tensor_tensor(wh2sb, wh2sb, ph3, OP.add)
            rsb_ps = pps.tile([128, R], F32, name=f"rsb{h}", tag="h7")
            nc.tensor.matmul(rsb_ps, lhsT=ones128, rhs=rsA, start=True, stop=True)
            rb = p2s.tile([128, R], F32, tag="rb", bufs=1)
            nc.vector.reciprocal(rb, rsb_ps)

            hstage = p2s.tile([128, 8, R], F16, name=f"hstage{h}", tag="hstage",
                              bufs=1)
            for os in range(8):
                eng = nc.gpsimd if os % 3 == 2 else nc.vector
                eng.tensor_tensor(hstage[:, os, :], hsb[os], rb, OP.mult)

            def _elu_tail(os, h=h, hstage=hstage):
                mn = p2w.tile([128, R], F16, tag="u2f")
                nc.vector.tensor_scalar_min(mn, hstage[:, os, :], 0.0)
                ex = p2w.tile([128, R], F16, tag="ex")
                nc.scalar.activation(ex, mn, AF.Exp)
                nc.vector.scalar_tensor_tensor(
                    out=xcatT[:, h * 8 + os, :], in0=ex, scalar=-1.0,
                    in1=hstage[:, os, :], op0=OP.add, op1=OP.max)

        for os in range(8):
            _elu_tail(os)

        # =============== phase 3 tail: heads 2-3 + combine ===============
        wh2T_ps = pps.tile([BIT, R], F32, tag="h2")
        for hp in (2, 3):
            for kk in range(8):
                nc.tensor.matmul(
                    wh2T_ps, lhsT=wob[:, hp * 8 + kk, :],
                    rhs=xcatT[:, hp * 8 + kk, :],
                    start=(hp == 2 and kk == 0), stop=(hp == 3 and kk == 7))
        wh2T = p2c.tile([BIT, R], F32)
        nc.vector.tensor_tensor(wh2T, wh2sb, wh2T_ps, OP.add)

        # staged ag2 payload [wh2 (64) | 1.0 | g2] per i-block, two halves:
        # the first collective flies while i-blocks 2-3 are still staging
        agst2 = [p2c.tile([128, 2, W2], BF16, name=f"agst{hf}")
                 for hf in range(2)]
        for hf in range(2):
            nc.vector.memset(agst2[hf][:, :, BIT:BIT + 1], 1.0)
        for i in range(IT):
            hf, sl = divmod(i, 2)
            tp_ps = pps.tile([128, BIT], F32, name=f"w2t{i}", tag="h4")
            nc.tensor.transpose(tp_ps, wh2T[:, i * 128:(i + 1) * 128],
                                ident[:BIT, :BIT])
            nc.vector.tensor_copy(agst2[hf][:, sl, :BIT], tp_ps)
            g2c = p2w.tile([128, 1], F32, tag="g2c")
            scratch2 = p2w.tile([128, BIT], F32, tag="scratch2")
            nc.vector.scalar_tensor_tensor(
                out=scratch2, in0=tp_ps, scalar=0.0, in1=a2o_b,
                op0=OP.bypass, op1=OP.mult, accum_out=g2c)
            nc.vector.tensor_copy(agst2[hf][:, sl, BIT + 1:W2], g2c)
            if sl == 1:
                nc.sync.dma_start(
                    ag2_in2[hf].rearrange("p (i z) -> p i z", i=2), agst2[hf])
                nc.gpsimd.collective_compute(
                    "AllGather", OP.bypass, ins=[ag2_in2[hf].opt()],
                    outs=[ag2_out2[hf].opt()], replica_groups=rg)

        # g1 logit table chain runs during the collectives
        g1T_ps = pps.tile([1, R], F32, tag="h3")
        nc.tensor.matmul(g1T_ps, lhsT=a1o_col, rhs=wh2T, start=True, stop=True)
        g1T = p2c.tile([1, R], F32)
        nc.vector.tensor_copy(g1T, g1T_ps)
        g1b_ps = pps.tile([128, R], F32, tag="h5")
        nc.tensor.matmul(g1b_ps, lhsT=ones_row, rhs=g1T, start=True, stop=True)
        # factored row table exp((1-a)*g1[r]); the exp(a*g1[r]) factor
        # cancels in the output-layer softmax normalization
        E1r = p2c.tile([128, R], BF16)
        nc.scalar.activation(E1r, g1b_ps, AF.Exp, scale=1.0 - ALPHA)

        # =============== phase 4: output attention ===============
        # pull each gathered half into SBUF as its collective lands; exps of
        # the packed g2 columns per half so the i 0-1 chunks start first
        G2o_all = p2c.tile([128, NC, IT, 1], F32)
        G2oa_all = p2c.tile([128, NC, IT, 1], F32)
        for hf in range(2):
            isl = slice(2 * hf, 2 * hf + 2)
            ag2v = ag2_out2[hf].rearrange("(c p) (i z) -> p c i z", c=NC, i=2)
            eng = nc.gpsimd if hf == 0 else nc.sync
            eng.dma_start(w2all[:, :, isl], ag2v)
            nc.scalar.activation(G2o_all[:, :, isl],
                                 w2all[:, :, isl, BIT + 1:W2], AF.Exp)
            nc.scalar.activation(G2oa_all[:, :, isl],
                                 w2all[:, :, isl, BIT + 1:W2], AF.Exp,
                                 scale=ALPHA)
        ht2_ps = pps.tile([BIT + 1, R], F32, tag="h6")
        for ihalf in range(2):
            for c in range(NC):
                for i in (2 * ihalf, 2 * ihalf + 1):
                    jt = c * IT + i
                    m = p2p.tile([128, R], BF16, tag="m")
                    nc.vector.tensor_scalar(
                        out=m, in0=E1r, scalar1=G2o_all[:, c, i, :],
                        scalar2=G2oa_all[:, c, i, :], op0=OP.mult, op1=OP.max)
                    u2 = p2p.tile([128, R], BF16, tag="u")
                    eng = nc.gpsimd if jt % 3 == 0 else nc.vector
                    eng.tensor_tensor(u2, m, adjT[:, jt, :], OP.mult)
                    nc.tensor.matmul(
                        ht2_ps, lhsT=w2all[:, c, i, :BIT + 1], rhs=u2,
                        start=(ihalf == 0 and c == 0 and i == 0),
                        stop=(ihalf == 1 and c == NC - 1 and i == 3))

        # transpose [65, R] (incl the row-sum row), then the row scale is
        # per-partition: one reciprocal + tanh(scale*in) per i-block.
        # separate PSUM tags per i so the four chains pipeline.
        ht2s = p2c.tile([BIT + 1, R], F32)
        ot_tags = ("h1", "h3", "h4", "h5")
        for i in range(IT):
            nc.vector.tensor_copy(ht2s[:, i * 128:(i + 1) * 128],
                                  ht2_ps[:, i * 128:(i + 1) * 128])
            tp_ps = pps.tile([128, BIT + 1], F32, name=f"ot{i}", tag=ot_tags[i])
            nc.tensor.transpose(tp_ps, ht2s[:, i * 128:(i + 1) * 128],
                                ident[:BIT + 1, :BIT + 1])
            rbc = p2w.tile([128, 1], F32, tag="rbc", bufs=2)
            nc.vector.reciprocal(rbc, tp_ps[:, BIT:BIT + 1])
            ob = p2w.tile([128, BIT], F32, tag="ob", bufs=2)
            nc.scalar.activation(ob, tp_ps[:, :BIT], AF.Tanh, scale=rbc)
            nc.sync.dma_start(out_d[i * 128:(i + 1) * 128, :], ob)

    _split_excess_waits(nc, max_waits=1)
    return nc


_CACHED = None


def _get_program():
    global _CACHED
    if _CACHED is None:
        _CACHED = build_program()
    return _CACHED


def _interleave(a, kt):
    """[kt*128, free...] -> [128, kt, free...] partition-major."""
    return np.ascontiguousarray(
        a.reshape(kt, 128, *a.shape[1:]).transpose(1, 0, *range(2, a.ndim + 1)))


def make_in_maps(x, adj, W, a1, a2, W_out, a1_out, a2_out):
    import ml_dtypes
    xT = np.ascontiguousarray(x.T)
    adjT_bf = adj.T.astype(ml_dtypes.bfloat16)
    # host-side tiny logit GEMM: f1/f2 halves for all heads, fp32
    # B = [W[h] @ a1[h] (4 cols) | W[h] @ a2[h] (4 cols)]
    B = np.concatenate(
        [np.stack([W[h] @ a1[h] for h in range(NHEADS)], axis=1),
         np.stack([W[h] @ a2[h] for h in range(NHEADS)], axis=1)],
        axis=1).astype(np.float32)
    F = x.astype(np.float64) @ B.astype(np.float64)   # [N, 8]
    f1 = np.ascontiguousarray(F[:, :NHEADS].T)    # [H, N]
    f2 = np.ascontiguousarray(F[:, NHEADS:].T)    # [H, N]
    E1 = np.exp(f1).astype(ml_dtypes.bfloat16)
    E1a = np.exp(ALPHA * f1).astype(ml_dtypes.bfloat16)
    # f2 exps in [128, H, IT, NC] partition-major j-tile layout
    def _f2sh(a):
        return np.ascontiguousarray(
            a.reshape(NHEADS, NC, IT, 128).transpose(3, 0, 2, 1)
        ).astype(np.float32)
    G2_sh = _f2sh(np.exp(f2 + BIAS_LN))
    G2a_sh = np.ascontiguousarray(
        np.exp(ALPHA * f2 + BIAS_LN).reshape(NHEADS, NC, IT, 128)
        .transpose(3, 0, 2, 1)).astype(np.float32)
    # W interleaved: [h, 128, KT, NHID]; fp8 e4m3 scaled by WS (|v| <= 240
    # keeps e4m3 and e4m3fn bit-identical)
    if PH1_FP8:
        def _q1(a, s):
            return np.clip(a * s, -240.0, 240.0).astype(ml_dtypes.float8_e4m3)
    else:
        def _q1(a, s):
            return a.astype(np.float16)
    W_sh = _q1(np.ascontiguousarray(
        W.reshape(NHEADS, KT, 128, NHID).transpose(0, 2, 1, 3)), WS)
    Wo_sh = _interleave(W_out, KT).astype(np.float16)
    in_maps = []
    for d in range(NC):
        cols = slice(d * R, (d + 1) * R)
        in_maps.append({
            "x_sh": _q1(_interleave(np.ascontiguousarray(xT[:, cols]), KT), XS),
            "W_sh": W_sh,
            "adj_sh": _interleave(np.ascontiguousarray(adjT_bf[:, cols]), JT),
            "Wo_sh": Wo_sh,
            "E1_sh": np.ascontiguousarray(np.broadcast_to(
                E1[None, :, d * R:(d + 1) * R], (128, NHEADS, R))),
            "E1a_sh": np.ascontiguousarray(np.broadcast_to(
                E1a[None, :, d * R:(d + 1) * R], (128, NHEADS, R))),
            "G2_sh": G2_sh, "G2a_sh": G2a_sh,
            "a1_out": a1_out, "a2_out": a2_out,
        })
    return in_maps


def kernel(x, adj, W, a1, a2, W_out, a1_out, a2_out, _trace=False):
    nc = _get_program()
    in_maps = make_in_maps(np.asarray(x, np.float32), np.asarray(adj, np.float32),
                           np.asarray(W, np.float32), np.asarray(a1, np.float32),
                           np.asarray(a2, np.float32), np.asarray(W_out, np.float32),
                           np.asarray(a1_out, np.float32),
                           np.asarray(a2_out, np.float32))
    res = bass_utils.run_bass_kernel_spmd(
        nc, in_maps, core_ids=list(range(NC)), trace=_trace)
    out = np.concatenate([res.results[d]["out_rows"] for d in range(NC)], axis=0)
    if _trace:
        kernel.last_exec_time_ns = res.exec_time_ns
        kernel.last_results = res
    return out



# revision 57
# speedup vs baseline: 1.0016x; 1.0016x over previous
"""Trainium2 Bass kernel for nn_GATNet_IMG (dense 2-layer GAT, N=4096).

Sharding: 1D row-parallel over the node dim across 8 NeuronCores.
Each core computes Wh for its 512 rows (all 4 heads), AllGathers Wh
per head (overlapped with the next head's matmuls), then computes its
[512, 4096] attention block per head with a fused masked softmax (no
NxN matrix ever hits HBM), aggregates h^T = Wh^T @ u on TensorE, and
repeats the same pattern for the output attention layer.

Key design points:
  - both attention-aggregate matmul operands are BF16 (mixed
    fp16/bf16 streams the PE at half rate); fp8 DoubleRow for phase 1
    is implemented behind PH1_FP8 but off — it computes exactly
    (verified vs host) yet the e4m3 input quantization alone costs
    ~5e-2 final rel err on the graded inputs, over the 2e-2 gate
  - phase-3 k-tiles run incrementally per head inside the phase-2
    PSUM-drain windows; the ag2 gather is split into two collectives
    so the first half of phase 4 overlaps the second collective
  - exp factorization: exp(leaky(f1+f2)+c) = max(E1[i]G2[j],
    E1a[i]G2a[j]) with E1=exp(f1), G2=exp(f2+c) etc., so the per-tile
    work is one ACT per-partition-scale multiply plus one fused DVE
    scalar_tensor_tensor (mult+max) instead of two full exps
  - f1/f2 logit halves precomputed on host in fp32 (tiny GEMM), so
    logit precision is independent of the big-GEMM compute dtype
  - unnormalized attention + broadcast row-sum via all-ones matmul;
    normalization is a fast-approx reciprocal postscale
  - elu(x) == max(exp(min(x, 0)) - 1, x)           (exact)
  - ln(2^-30) folded into the exp bias keeps row sums in range
  - dual DMA queues: bulk streams on SP, latency-critical prefetches
    (next-phase Wh blocks, adj) on the Activation HWDGE queue
  - host-side sharding pre-transposes x/adj and interleaves heavy
    streams into [128, ktiles, free] partition-major layouts
"""

import math
from contextlib import ExitStack

import numpy as np

import concourse.bass as bass
import concourse.mybir as mybir
import concourse.tile as tile
from concourse import bass_utils
from concourse.masks import make_identity

F32 = mybir.dt.float32
F16 = mybir.dt.float16
BF16 = mybir.dt.bfloat16
F8 = mybir.dt.float8e4
AF = mybir.ActivationFunctionType
OP = mybir.AluOpType
DR = mybir.MatmulPerfMode.DoubleRow

N = 4096
NFEAT = 4096
NHID = 1024
NHEADS = 4
BIT = 64
NC = 8
R = N // NC          # 512 attention rows per core
KT = NFEAT // 128    # 32 k tiles
JT = N // 128        # 32 node-column tiles
IT = R // 128        # 4 row tiles per core
ALPHA = 0.2
BIAS_LN = -30.0 * math.log(2.0)   # ln(2^-30) folded into exp bias (layer 1)
W2 = BIT + 2         # packed ag2 row: 64 bf16 Wh2 + 1.0 + bf16 g2
PH1_FP8 = False       # phase-1 GEMM in fp8 e4m3 DoubleRow vs fp16
XS = 8.0             # fp8 phase-1 input scales (x*XS, W*WS; both <= 240,
WS = 128.0           # and product scale small enough that Wh*XS*WS stays
                     # far from the fp16 range in case the PE's DoubleRow
                     # path carries reduced-precision partials)
INV_SCALE = 1.0 / (XS * WS) if PH1_FP8 else 1.0


def _split_excess_waits(nc, max_waits=1):
    """walrus codegen rejects >max_waits sync-wait commands per instruction;
    push excess waits onto preceding same-engine NoOps."""
    n_fixed = 0
    for f in nc.m.functions:
        for b in f.blocks:
            new_insts = []
            changed = False
            for inst in b.instructions:
                si = getattr(inst, "sync_info", None)
                if si is not None and si.on_wait and len(si.on_wait) > max_waits:
                    waits = list(si.on_wait)
                    excess, keep = waits[:-max_waits], waits[-max_waits:]
                    for ci in range(0, len(excess), max_waits):
                        nop = mybir.InstNoOp(
                            name=f"{inst.name}-ws{ci}",
                            sync_info=mybir.SyncInfo(
                                on_wait=excess[ci:ci + max_waits], on_update=[]
                            ),
                            bass_nofuse=True,
                            engine=inst.engine,
                        )
                        new_insts.append(nop)
                    inst.sync_info = mybir.SyncInfo(
                        on_wait=keep, on_update=list(si.on_update or [])
                    )
                    n_fixed += 1
                    changed = True
                new_insts.append(inst)
            if changed:
                insts = b.instructions
                try:
                    b.instructions = new_insts
                except Exception:
                    while len(insts):
                        insts.pop()
                    for i in new_insts:
                        insts.append(i)
    return n_fixed


def build_program():
    nc = bass.Bass("TRN2", target_bir_lowering=False, debug=False, num_devices=NC)

    PH1_DT = F8 if PH1_FP8 else F16
    # host-interleaved inputs: [128 partitions, ktiles, free]
    x_d = nc.dram_tensor("x_sh", [128, KT, R], PH1_DT, kind="ExternalInput").ap()
    W_d = nc.dram_tensor("W_sh", [NHEADS, 128, KT, NHID], PH1_DT,
                         kind="ExternalInput").ap()
    adj_d = nc.dram_tensor("adj_sh", [128, JT, R], BF16, kind="ExternalInput").ap()
    wo_d = nc.dram_tensor("Wo_sh", [128, KT, BIT], F16, kind="ExternalInput").ap()
    E1_d = nc.dram_tensor("E1_sh", [128, NHEADS, R], BF16,
                          kind="ExternalInput").ap()
    E1a_d = nc.dram_tensor("E1a_sh", [128, NHEADS, R], BF16,
                           kind="ExternalInput").ap()
    G2_d = nc.dram_tensor("G2_sh", [128, NHEADS, IT, NC], F32,
                          kind="ExternalInput").ap()
    G2a_d = nc.dram_tensor("G2a_sh", [128, NHEADS, IT, NC], F32,
                           kind="ExternalInput").ap()
    a1o_d = nc.dram_tensor("a1_out", [BIT], F32, kind="ExternalInput").ap()
    a2o_d = nc.dram_tensor("a2_out", [BIT], F32, kind="ExternalInput").ap()
    out_d = nc.dram_tensor("out_rows", [R, BIT], F32, kind="ExternalOutput").ap()

    # collective bounce buffers; Wh per head so each head's allgather overlaps
    # the next head's phase-1 compute.
    ag1_in = [nc.dram_tensor(f"ag1_in{h}", [128, IT * NHID], BF16).ap()
              for h in range(NHEADS)]
    ag1_out = [nc.dram_tensor(f"ag1_out{h}", [NC * 128, IT * NHID], BF16,
                              addr_space="Shared").ap() for h in range(NHEADS)]
    # ag2 in two halves (i-blocks 0-1 / 2-3) so the first collective and
    # the first half of phase 4 overlap the second collective
    ag2_in2 = [nc.dram_tensor(f"ag2_in{half}", [128, 2 * W2], BF16).ap()
               for half in range(2)]
    ag2_out2 = [nc.dram_tensor(f"ag2_out{half}", [NC * 128, 2 * W2], BF16,
                               addr_space="Shared").ap() for half in range(2)]

    rg = [list(range(NC))]

    with tile.TileContext(nc) as tc, ExitStack() as ctx:
        cp = ctx.enter_context(tc.tile_pool(name="const", bufs=1))
        ident = cp.tile([128, 128], F32)
        make_identity(nc, ident)
        ones128 = cp.tile([128, 128], F32)
        nc.vector.memset(ones128, 1.0)
        a1o_col = cp.tile([BIT, 1], F32)
        a2o_b = cp.tile([128, BIT], F32)
        ones_row = cp.tile([1, 128], F32)
        nc.vector.memset(ones_row, 1.0)
        # logit tables: host-precomputed exps of the f1/f2 halves
        G2 = cp.tile([128, NHEADS, IT, NC], F32)
        G2a = cp.tile([128, NHEADS, IT, NC], F32)
        E1b = cp.tile([128, NHEADS, R], BF16)
        E1ab = cp.tile([128, NHEADS, R], BF16)
        # adjacency mask, resident for both attention layers
        adjT = cp.tile([128, JT, R], BF16)
        wob = cp.tile([128, KT, BIT], F16)
        # phase-2 head-0 Wh prefetch (filled mid-phase-1 via the ACT queue)
        whtA = [cp.tile([128, IT, NHID], BF16, name=f"whtA{c}") for c in range(2)]
        # phase-4 Wh2 blocks, all 8 chunks resident (4.2 KiB/partition)
        w2all = cp.tile([128, NC, IT, W2], BF16)

        # =============== phase 1: Wh = x @ W[h] ===============
        with tc.tile_pool(name="p0", bufs=1) as p0, \
             tc.tile_pool(name="p1s", bufs=4) as p1s, \
             tc.tile_pool(name="p1ps", bufs=1, space="PSUM") as p1ps, \
             tc.tile_pool(name="p1d", bufs=3) as p1d:
            xp1 = p0.tile([128, KT, R], PH1_DT)
            for q in range(8):
                nc.sync.dma_start(xp1[:, q * 4:(q + 1) * 4, :],
                                  x_d[:, q * 4:(q + 1) * 4, :])
            for h in range(NHEADS):
                ps = [[p1ps.tile([128, 512], F32, name=f"ps_{h}_{i}_{oh}",
                                 tag=f"ps{i}{oh}") for oh in range(2)]
                      for i in range(IT)]
                for kb in range(4):
                    wres = p1s.tile([128, 8, NHID], PH1_DT, tag="wres")
                    if h == 0:
                        if kb == 0:
                            nc.scalar.dma_start(
                                wres[:, :2, :], W_d[0, :, 0:2, :])
                            nc.scalar.dma_start(
                                wres[:, 2:, :], W_d[0, :, 2:8, :])
                            # tiny phase-3 vectors behind the critical W chunk
                            nc.scalar.dma_start(
                                a1o_col, a1o_d.rearrange("(b one) -> b one",
                                                         one=1))
                            nc.scalar.dma_start(
                                a2o_b, a2o_d.rearrange(
                                    "(one b) -> one b",
                                    one=1).to_broadcast([128, BIT]))
                        else:
                            nc.scalar.dma_start(
                                wres, W_d[0, :, kb * 8:(kb + 1) * 8, :])
                        if kb == 3:
                            # logit tables + adj behind all four W chunks
                            nc.scalar.dma_start(E1b, E1_d)
                            nc.scalar.dma_start(E1ab, E1a_d)
                            nc.scalar.dma_start(G2, G2_d)
                            nc.scalar.dma_start(G2a, G2a_d)
                            nc.scalar.dma_start(adjT, adj_d)
                    else:
                        nc.sync.dma_start(wres, W_d[h, :, kb * 8:(kb + 1) * 8, :])
                    if PH1_FP8:
                        for kk2 in range(4):
                            k2 = kb * 4 + kk2
                            for i in range(IT):
                                for oh in range(2):
                                    nc.tensor.matmul(
                                        ps[i][oh],
                                        lhsT=xp1[:, 2 * k2:2 * k2 + 2,
                                                 i * 128:(i + 1) * 128],
                                        rhs=wres[:, 2 * kk2:2 * kk2 + 2,
                                                 oh * 512:(oh + 1) * 512],
                                        start=(k2 == 0),
                                        stop=(k2 == KT // 2 - 1),
                                        perf_mode=DR,
                                    )
                    else:
                        for kk in range(8):
                            k = kb * 8 + kk
                            for i in range(IT):
                                for oh in range(2):
                                    nc.tensor.matmul(
                                        ps[i][oh],
                                        lhsT=xp1[:, k, i * 128:(i + 1) * 128],
                                        rhs=wres[:, kk,
                                                 oh * 512:(oh + 1) * 512],
                                        start=(k == 0), stop=(k == KT - 1),
                                    )
                if h == 0:
                    nc.scalar.dma_start(wob, wo_d)
                for i in range(IT):
                    wh_sb = p1d.tile([128, NHID], BF16, tag="wh_sb")
                    nc.vector.tensor_scalar_mul(wh_sb[:, :512], ps[i][0],
                                                INV_SCALE)
                    nc.scalar.mul(wh_sb[:, 512:], ps[i][1], INV_SCALE)
                    # scalar queue: the sync queue stays a pure x/W stream,
                    # so the next head's W chunks are never stuck behind
                    # these drain-gated writes
                    nc.scalar.dma_start(
                        ag1_in[h][:, i * NHID:(i + 1) * NHID], wh_sb)
                # allgather this head's Wh while later heads compute
                nc.gpsimd.collective_compute(
                    "AllGather", OP.bypass, ins=[ag1_in[h].opt()],
                    outs=[ag1_out[h].opt()], replica_groups=rg)
                if h == 1:
                    # prefetch head-0's first attention Wh blocks on the Pool
                    # queue: Pool is idle all of phase 1, so its blocking
                    # wait on ag1[0] completion head-of-line-blocks nothing
                    for c in range(2):
                        nc.gpsimd.dma_start(
                            whtA[c], ag1_out[0][c * 128:(c + 1) * 128, :].rearrange(
                                "p (i o) -> p i o", i=IT))

        # =============== phase 2: attention + aggregate, per head ===============
        p2c = ctx.enter_context(tc.tile_pool(name="p2c", bufs=1))
        xcatT = p2c.tile([128, KT, R], F16)

        pps = ctx.enter_context(tc.tile_pool(name="pps", bufs=1, space="PSUM"))
        p2s = ctx.enter_context(tc.tile_pool(name="p2s", bufs=2))
        p2w = ctx.enter_context(tc.tile_pool(name="p2w", bufs=2))
        p2p = ctx.enter_context(tc.tile_pool(name="p2p", bufs=4))
        # phase-3 running sum, fed incrementally as each head's xcat lands
        wh2sb = p2c.tile([BIT, R], F32)

        for h in range(NHEADS):
            rsA = p2s.tile([128, R], F32, tag="rsA")
            nc.gpsimd.memset(rsA, 0.0)

            hps = [pps.tile([128, R], F32, name=f"hps{h}_{os}", tag=f"h{os}")
                   for os in range(8)]
            for c in range(NC):
                if h == 0 and c < 2:
                    wht4 = whtA[c]
                else:
                    wht4 = p2w.tile([128, IT, NHID], BF16, tag="wht", bufs=3)
                    nc.sync.dma_start(
                        wht4, ag1_out[h][c * 128:(c + 1) * 128, :].rearrange(
                            "p (i o) -> p i o", i=IT))
                for i in range(IT):
                    # previous head's elu tails, mid-chunk so the DVE ops
                    # don't delay this chunk's first u tiles; none on the
                    # last chunk, whose final u tiles gate the head's tail
                    if h > 0 and ((c < 6 and i == 2) or
                                  (c == 6 and i in (1, 3))):
                        _elu_tail(c if c < 6 else 6 + (i == 3))
                    jt = c * IT + i
                    e2 = p2p.tile([128, R], BF16, tag="e2")
                    nc.scalar.mul(e2, E1ab[:, h, :], G2a[:, h, i, c:c + 1])
                    m = p2p.tile([128, R], BF16, tag="m")
                    if jt % 2 == 0:
                        nc.vector.scalar_tensor_tensor(
                            out=m, in0=E1b[:, h, :], scalar=G2[:, h, i, c:c + 1],
                            in1=e2, op0=OP.mult, op1=OP.max)
                    else:
                        e1 = p2p.tile([128, R], BF16, tag="e1")
                        nc.scalar.mul(e1, E1b[:, h, :], G2[:, h, i, c:c + 1])
                        nc.vector.tensor_tensor(m, e1, e2, OP.max)
                    u = p2p.tile([128, R], BF16, tag="u")
                    nc.vector.tensor_tensor(u, m, adjT[:, jt, :], OP.mult)
                    nc.gpsimd.tensor_tensor(rsA, rsA, u, OP.add)
                    for os in range(8):
                        nc.tensor.matmul(
                            hps[os], lhsT=wht4[:, i, os * 128:(os + 1) * 128],
                            rhs=u, start=(jt == 0), stop=(jt == JT - 1))

            # eager PSUM drain (banks freed for the next head asap), then the
            # broadcast row-sum + approx-reciprocal normalization chain
            hsb = [p2s.tile([128, R], F32, name=f"hsb{h}_{os}", tag=f"hsb{os}",
                            bufs=1)
                   for os in range(8)]
            # DVE drains in stop order (os 0 stops first in jt=31's os-loop);
            # ACT drains os 7 first so the rowsum matmul (which WARs on
            # hsb[7]) unblocks as early as possible
            for os in (0, 2, 4, 6):
                nc.vector.tensor_copy(hsb[os], hps[os])
            for os in (7, 1, 3, 5):
                nc.scalar.copy(hsb[os], hps[os])
            if h in (1, 2):
                # heads 0/1's phase-3 k-tiles fill the drain windows: PSUM
                # h7 frees first and the PE would otherwise idle on drains.
                # Heads 2-3 stay in the tail so the PE has work to chew
                # while head 3's elu chain runs.
                hp = h - 1
                ph3 = pps.tile([BIT, R], F32, name=f"ph3_{hp}", tag="h7")
                for kk in range(8):
                    nc.tensor.matmul(
                        ph3, lhsT=wob[:, hp * 8 + kk, :],
                        rhs=xcatT[:, hp * 8 + kk, :],
                        start=(kk == 0), stop=(kk == 7))
                if hp == 0:
                    nc.scalar.copy(wh2sb, ph3)
                else:
                    nc.vector.tensor_tensor(wh2sb, wh2sb, ph3, OP.add)
            rsb_ps = pps.tile([128, R], F32, name=f"rsb{h}", tag="h7")
            nc.tensor.matmul(rsb_ps, lhsT=ones128, rhs=rsA, start=True, stop=True)
            rb = p2s.tile([128, R], F32, tag="rb", bufs=1)
            nc.vector.reciprocal(rb, rsb_ps)

            hstage = p2s.tile([128, 8, R], F16, name=f"hstage{h}", tag="hstage",
                              bufs=1)
            for os in range(8):
                eng = nc.gpsimd if os % 3 == 2 else nc.vector
                eng.tensor_tensor(hstage[:, os, :], hsb[os], rb, OP.mult)

            def _elu_tail(os, h=h, hstage=hstage):
                mn = p2w.tile([128, R], F16, tag="u2f")
                nc.vector.tensor_scalar_min(mn, hstage[:, os, :], 0.0)
                ex = p2w.tile([128, R], F16, tag="ex")
                nc.scalar.activation(ex, mn, AF.Exp)
                nc.vector.scalar_tensor_tensor(
                    out=xcatT[:, h * 8 + os, :], in0=ex, scalar=-1.0,
                    in1=hstage[:, os, :], op0=OP.add, op1=OP.max)

        for os in range(8):
            _elu_tail(os)

        # =============== phase 3 tail: heads 2-3 + combine ===============
        wh2T_ps = pps.tile([BIT, R], F32, tag="h2")
        for hp in (2, 3):
            for kk in range(8):
                nc.tensor.matmul(
                    wh2T_ps, lhsT=wob[:, hp * 8 + kk, :],
                    rhs=xcatT[:, hp * 8 + kk, :],
                    start=(hp == 2 and kk == 0), stop=(hp == 3 and kk == 7))
        wh2T = p2c.tile([BIT, R], F32)
        nc.vector.tensor_tensor(wh2T, wh2sb, wh2T_ps, OP.add)

        # staged ag2 payload [wh2 (64) | 1.0 | g2] per i-block, two halves:
        # the first collective flies while i-blocks 2-3 are still staging
        agst2 = [p2c.tile([128, 2, W2], BF16, name=f"agst{hf}")
                 for hf in range(2)]
        for hf in range(2):
            nc.vector.memset(agst2[hf][:, :, BIT:BIT + 1], 1.0)
        for i in range(IT):
            hf, sl = divmod(i, 2)
            tp_ps = pps.tile([128, BIT], F32, name=f"w2t{i}", tag="h4")
            nc.tensor.transpose(tp_ps, wh2T[:, i * 128:(i + 1) * 128],
                                ident[:BIT, :BIT])
            nc.vector.tensor_copy(agst2[hf][:, sl, :BIT], tp_ps)
            g2c = p2w.tile([128, 1], F32, tag="g2c")
            scratch2 = p2w.tile([128, BIT], F32, tag="scratch2")
            nc.vector.scalar_tensor_tensor(
                out=scratch2, in0=tp_ps, scalar=0.0, in1=a2o_b,
                op0=OP.bypass, op1=OP.mult, accum_out=g2c)
            nc.vector.tensor_copy(agst2[hf][:, sl, BIT + 1:W2], g2c)
            if sl == 1:
                nc.sync.dma_start(
                    ag2_in2[hf].rearrange("p (i z) -> p i z", i=2), agst2[hf])
                nc.gpsimd.collective_compute(
                    "AllGather", OP.bypass, ins=[ag2_in2[hf].opt()],
                    outs=[ag2_out2[hf].opt()], replica_groups=rg)

        # g1 logit table chain runs during the collectives
        g1T_ps = pps.tile([1, R], F32, tag="h3")
        nc.tensor.matmul(g1T_ps, lhsT=a1o_col, rhs=wh2T, start=True, stop=True)
        g1T = p2c.tile([1, R], F32)
        nc.vector.tensor_copy(g1T, g1T_ps)
        g1b_ps = pps.tile([128, R], F32, tag="h5")
        nc.tensor.matmul(g1b_ps, lhsT=ones_row, rhs=g1T, start=True, stop=True)
        # factored row table exp((1-a)*g1[r]); the exp(a*g1[r]) factor
        # cancels in the output-layer softmax normalization
        E1r = p2c.tile([128, R], BF16)
        nc.scalar.activation(E1r, g1b_ps, AF.Exp, scale=1.0 - ALPHA)

        # =============== phase 4: output attention ===============
        # pull each gathered half into SBUF as its collective lands; exps of
        # the packed g2 columns per half so the i 0-1 chunks start first
        G2o_all = p2c.tile([128, NC, IT, 1], F32)
        G2oa_all = p2c.tile([128, NC, IT, 1], F32)
        for hf in range(2):
            isl = slice(2 * hf, 2 * hf + 2)
            ag2v = ag2_out2[hf].rearrange("(c p) (i z) -> p c i z", c=NC, i=2)
            eng = nc.gpsimd if hf == 0 else nc.sync
            eng.dma_start(w2all[:, :, isl], ag2v)
            nc.scalar.activation(G2o_all[:, :, isl],
                                 w2all[:, :, isl, BIT + 1:W2], AF.Exp)
            nc.scalar.activation(G2oa_all[:, :, isl],
                                 w2all[:, :, isl, BIT + 1:W2], AF.Exp,
                                 scale=ALPHA)
        ht2_ps = pps.tile([BIT + 1, R], F32, tag="h6")
        for ihalf in range(2):
            for c in range(NC):
                for i in (2 * ihalf, 2 * ihalf + 1):
                    jt = c * IT + i
                    m = p2p.tile([128, R], BF16, tag="m")
                    nc.vector.tensor_scalar(
                        out=m, in0=E1r, scalar1=G2o_all[:, c, i, :],
                        scalar2=G2oa_all[:, c, i, :], op0=OP.mult, op1=OP.max)
                    u2 = p2p.tile([128, R], BF16, tag="u")
                    eng = nc.gpsimd if jt % 3 == 0 else nc.vector
                    eng.tensor_tensor(u2, m, adjT[:, jt, :], OP.mult)
                    nc.tensor.matmul(
                        ht2_ps, lhsT=w2all[:, c, i, :BIT + 1], rhs=u2,
                        start=(ihalf == 0 and c == 0 and i == 0),
                        stop=(ihalf == 1 and c == NC - 1 and i == 3))

        # transpose [65, R] (incl the row-sum row), then the row scale is
        # per-partition: one reciprocal + tanh(scale*in) per i-block.
        # separate PSUM tags per i so the four chains pipeline.
        ht2s = p2c.tile([BIT + 1, R], F32)
        ot_tags = ("h1", "h3", "h4", "h5")
        for i in range(IT):
            nc.vector.tensor_copy(ht2s[:, i * 128:(i + 1) * 128],
                                  ht2_ps[:, i * 128:(i + 1) * 128])
            tp_ps = pps.tile([128, BIT + 1], F32, name=f"ot{i}", tag=ot_tags[i])
            nc.tensor.transpose(tp_ps, ht2s[:, i * 128:(i + 1) * 128],
                                ident[:BIT + 1, :BIT + 1])
            rbc = p2w.tile([128, 1], F32, tag="rbc", bufs=2)
            nc.vector.reciprocal(rbc, tp_ps[:, BIT:BIT + 1])
            ob = p2w.tile([128, BIT], F32, tag="ob", bufs=2)
            nc.scalar.activation(ob, tp_ps[:, :BIT], AF.Tanh, scale=rbc)
            nc.sync.dma_start(out_d[i * 128:(i + 1) * 128, :], ob)

    _split_excess_waits(nc, max_waits=1)
    return nc


_CACHED = None


def _get_program():
    global _CACHED
    if _CACHED is None:
        _CACHED = build_program()
    return _CACHED


def _interleave(a, kt):
    """[kt*128, free...] -> [128, kt, free...] partition-major."""
    return np.ascontiguousarray(
        a.reshape(kt, 128, *a.shape[1:]).transpose(1, 0, *range(2, a.ndim + 1)))


def make_in_maps(x, adj, W, a1, a2, W_out, a1_out, a2_out):
    import ml_dtypes
    xT = np.ascontiguousarray(x.T)
    adjT_bf = adj.T.astype(ml_dtypes.bfloat16)
    # host-side tiny logit GEMM: f1/f2 halves for all heads, fp32
    # B = [W[h] @ a1[h] (4 cols) | W[h] @ a2[h] (4 cols)]
    B = np.concatenate(
        [np.stack([W[h] @ a1[h] for h in range(NHEADS)], axis=1),
         np.stack([W[h] @ a2[h] for h in range(NHEADS)], axis=1)],
        axis=1).astype(np.float32)
    F = x.astype(np.float64) @ B.astype(np.float64)   # [N, 8]
    f1 = np.ascontiguousarray(F[:, :NHEADS].T)    # [H, N]
    f2 = np.ascontiguousarray(F[:, NHEADS:].T)    # [H, N]
    E1 = np.exp(f1).astype(ml_dtypes.bfloat16)
    E1a = np.exp(ALPHA * f1).astype(ml_dtypes.bfloat16)
    # f2 exps in [128, H, IT, NC] partition-major j-tile layout
    def _f2sh(a):
        return np.ascontiguousarray(
            a.reshape(NHEADS, NC, IT, 128).transpose(3, 0, 2, 1)
        ).astype(np.float32)
    G2_sh = _f2sh(np.exp(f2 + BIAS_LN))
    G2a_sh = np.ascontiguousarray(
        np.exp(ALPHA * f2 + BIAS_LN).reshape(NHEADS, NC, IT, 128)
        .transpose(3, 0, 2, 1)).astype(np.float32)
    # W interleaved: [h, 128, KT, NHID]; fp8 e4m3 scaled by WS (|v| <= 240
    # keeps e4m3 and e4m3fn bit-identical)
    if PH1_FP8:
        def _q1(a, s):
            return np.clip(a * s, -240.0, 240.0).astype(ml_dtypes.float8_e4m3)
    else:
        def _q1(a, s):
            return a.astype(np.float16)
    W_sh = _q1(np.ascontiguousarray(
        W.reshape(NHEADS, KT, 128, NHID).transpose(0, 2, 1, 3)), WS)
    Wo_sh = _interleave(W_out, KT).astype(np.float16)
    in_maps = []
    for d in range(NC):
        cols = slice(d * R, (d + 1) * R)
        in_maps.append({
            "x_sh": _q1(_interleave(np.ascontiguousarray(xT[:, cols]), KT), XS),
            "W_sh": W_sh,
            "adj_sh": _interleave(np.ascontiguousarray(adjT_bf[:, cols]), JT),
            "Wo_sh": Wo_sh,
            "E1_sh": np.ascontiguousarray(np.broadcast_to(
                E1[None, :, d * R:(d + 1) * R], (128, NHEADS, R))),
            "E1a_sh": np.ascontiguousarray(np.broadcast_to(
                E1a[None, :, d * R:(d + 1) * R], (128, NHEADS, R))),
            "G2_sh": G2_sh, "G2a_sh": G2a_sh,
            "a1_out": a1_out, "a2_out": a2_out,
        })
    return in_maps


def kernel(x, adj, W, a1, a2, W_out, a1_out, a2_out, _trace=False):
    nc = _get_program()
    in_maps = make_in_maps(np.asarray(x, np.float32), np.asarray(adj, np.float32),
                           np.asarray(W, np.float32), np.asarray(a1, np.float32),
                           np.asarray(a2, np.float32), np.asarray(W_out, np.float32),
                           np.asarray(a1_out, np.float32),
                           np.asarray(a2_out, np.float32))
    res = bass_utils.run_bass_kernel_spmd(
        nc, in_maps, core_ids=list(range(NC)), trace=_trace)
    out = np.concatenate([res.results[d]["out_rows"] for d in range(NC)], axis=0)
    if _trace:
        kernel.last_exec_time_ns = res.exec_time_ns
        kernel.last_results = res
    return out



# revision 59
# speedup vs baseline: 1.0208x; 1.0192x over previous
"""Trainium2 Bass kernel for nn_GATNet_IMG (dense 2-layer GAT, N=4096).

Sharding: 1D row-parallel over the node dim across 8 NeuronCores.
Each core computes Wh for its 512 rows (all 4 heads), AllGathers Wh
per head (overlapped with the next head's matmuls), then computes its
[512, 4096] attention block per head with a fused masked softmax (no
NxN matrix ever hits HBM), aggregates h^T = Wh^T @ u on TensorE, and
repeats the same pattern for the output attention layer.

Key design points:
  - both attention-aggregate matmul operands are BF16 (mixed
    fp16/bf16 streams the PE at half rate); fp8 DoubleRow for phase 1
    is implemented behind PH1_FP8 but off — it computes exactly
    (verified vs host) yet the e4m3 input quantization alone costs
    ~5e-2 final rel err on the graded inputs, over the 2e-2 gate
  - phase-3 k-tiles run incrementally per head inside the phase-2
    PSUM-drain windows; the ag2 gather is split into two collectives
    so the first half of phase 4 overlaps the second collective
  - exp factorization: exp(leaky(f1+f2)+c) = max(E1[i]G2[j],
    E1a[i]G2a[j]) with E1=exp(f1), G2=exp(f2+c) etc., so the per-tile
    work is one ACT per-partition-scale multiply plus one fused DVE
    scalar_tensor_tensor (mult+max) instead of two full exps
  - f1/f2 logit halves precomputed on host in fp32 (tiny GEMM), so
    logit precision is independent of the big-GEMM compute dtype
  - unnormalized attention + broadcast row-sum via all-ones matmul;
    normalization is a fast-approx reciprocal postscale
  - elu(x) == max(exp(min(x, 0)) - 1, x)           (exact)
  - ln(2^-30) folded into the exp bias keeps row sums in range
  - dual DMA queues: bulk streams on SP, latency-critical prefetches
    (next-phase Wh blocks, adj) on the Activation HWDGE queue
  - host-side sharding pre-transposes x/adj and interleaves heavy
    streams into [128, ktiles, free] partition-major layouts
"""

import math
from contextlib import ExitStack

import numpy as np

import concourse.bass as bass
import concourse.mybir as mybir
import concourse.tile as tile
from concourse import bass_utils
from concourse.masks import make_identity

F32 = mybir.dt.float32
F16 = mybir.dt.float16
BF16 = mybir.dt.bfloat16
F8 = mybir.dt.float8e4
AF = mybir.ActivationFunctionType
OP = mybir.AluOpType
DR = mybir.MatmulPerfMode.DoubleRow

N = 4096
NFEAT = 4096
NHID = 1024
NHEADS = 4
BIT = 64
NC = 8
R = N // NC          # 512 attention rows per core
KT = NFEAT // 128    # 32 k tiles
JT = N // 128        # 32 node-column tiles
IT = R // 128        # 4 row tiles per core
ALPHA = 0.2
BIAS_LN = -30.0 * math.log(2.0)   # ln(2^-30) folded into exp bias (layer 1)
W2 = BIT + 2         # packed ag2 row: 64 bf16 Wh2 + 1.0 + bf16 g2
PH1_FP8 = False       # phase-1 GEMM in fp8 e4m3 DoubleRow vs fp16
XS = 8.0             # fp8 phase-1 input scales (x*XS, W*WS; both <= 240,
WS = 128.0           # and product scale small enough that Wh*XS*WS stays
                     # far from the fp16 range in case the PE's DoubleRow
                     # path carries reduced-precision partials)
INV_SCALE = 1.0 / (XS * WS) if PH1_FP8 else 1.0


def _split_excess_waits(nc, max_waits=1):
    """walrus codegen rejects >max_waits sync-wait commands per instruction;
    push excess waits onto preceding same-engine NoOps."""
    n_fixed = 0
    for f in nc.m.functions:
        for b in f.blocks:
            new_insts = []
            changed = False
            for inst in b.instructions:
                si = getattr(inst, "sync_info", None)
                if si is not None and si.on_wait and len(si.on_wait) > max_waits:
                    waits = list(si.on_wait)
                    excess, keep = waits[:-max_waits], waits[-max_waits:]
                    for ci in range(0, len(excess), max_waits):
                        nop = mybir.InstNoOp(
                            name=f"{inst.name}-ws{ci}",
                            sync_info=mybir.SyncInfo(
                                on_wait=excess[ci:ci + max_waits], on_update=[]
                            ),
                            bass_nofuse=True,
                            engine=inst.engine,
                        )
                        new_insts.append(nop)
                    inst.sync_info = mybir.SyncInfo(
                        on_wait=keep, on_update=list(si.on_update or [])
                    )
                    n_fixed += 1
                    changed = True
                new_insts.append(inst)
            if changed:
                insts = b.instructions
                try:
                    b.instructions = new_insts
                except Exception:
                    while len(insts):
                        insts.pop()
                    for i in new_insts:
                        insts.append(i)
    return n_fixed


def build_program():
    nc = bass.Bass("TRN2", target_bir_lowering=False, debug=False, num_devices=NC)

    PH1_DT = F8 if PH1_FP8 else F16
    # host-interleaved inputs: [128 partitions, ktiles, free]
    x_d = nc.dram_tensor("x_sh", [128, KT, R], PH1_DT, kind="ExternalInput").ap()
    W_d = nc.dram_tensor("W_sh", [NHEADS, 128, KT, NHID], PH1_DT,
                         kind="ExternalInput").ap()
    adj_d = nc.dram_tensor("adj_sh", [128, JT, R], BF16, kind="ExternalInput").ap()
    wo_d = nc.dram_tensor("Wo_sh", [128, KT, BIT], F16, kind="ExternalInput").ap()
    E1_d = nc.dram_tensor("E1_sh", [128, NHEADS, R], BF16,
                          kind="ExternalInput").ap()
    E1a_d = nc.dram_tensor("E1a_sh", [128, NHEADS, R], BF16,
                           kind="ExternalInput").ap()
    G2_d = nc.dram_tensor("G2_sh", [128, NHEADS, IT, NC], F32,
                          kind="ExternalInput").ap()
    G2a_d = nc.dram_tensor("G2a_sh", [128, NHEADS, IT, NC], F32,
                           kind="ExternalInput").ap()
    a1o_d = nc.dram_tensor("a1_out", [BIT], F32, kind="ExternalInput").ap()
    a2o_d = nc.dram_tensor("a2_out", [BIT], F32, kind="ExternalInput").ap()
    out_d = nc.dram_tensor("out_rows", [R, BIT], F32, kind="ExternalOutput").ap()

    # collective bounce buffers; Wh per head so each head's allgather overlaps
    # the next head's phase-1 compute.
    ag1_in = [nc.dram_tensor(f"ag1_in{h}", [128, IT * NHID], BF16).ap()
              for h in range(NHEADS)]
    ag1_out = [nc.dram_tensor(f"ag1_out{h}", [NC * 128, IT * NHID], BF16,
                              addr_space="Shared").ap() for h in range(NHEADS)]
    # ag2 in two halves (i-blocks 0-1 / 2-3) so the first collective and
    # the first half of phase 4 overlap the second collective
    ag2_in2 = [nc.dram_tensor(f"ag2_in{half}", [128, 2 * W2], BF16).ap()
               for half in range(2)]
    ag2_out2 = [nc.dram_tensor(f"ag2_out{half}", [NC * 128, 2 * W2], BF16,
                               addr_space="Shared").ap() for half in range(2)]

    rg = [list(range(NC))]

    with tile.TileContext(nc) as tc, ExitStack() as ctx:
        cp = ctx.enter_context(tc.tile_pool(name="const", bufs=1))
        ident = cp.tile([128, 128], F32)
        make_identity(nc, ident)
        ones128 = cp.tile([128, 128], F32)
        nc.vector.memset(ones128, 1.0)
        a1o_col = cp.tile([BIT, 1], F32)
        a2o_b = cp.tile([128, BIT], F32)
        ones_row = cp.tile([1, 128], F32)
        nc.vector.memset(ones_row, 1.0)
        # logit tables: host-precomputed exps of the f1/f2 halves
        G2 = cp.tile([128, NHEADS, IT, NC], F32)
        G2a = cp.tile([128, NHEADS, IT, NC], F32)
        E1b = cp.tile([128, NHEADS, R], BF16)
        E1ab = cp.tile([128, NHEADS, R], BF16)
        # adjacency mask, resident for both attention layers
        adjT = cp.tile([128, JT, R], BF16)
        wob = cp.tile([128, KT, BIT], F16)
        # phase-2 head-0 Wh prefetch (filled mid-phase-1 via the ACT queue)
        whtA = [cp.tile([128, IT, NHID], BF16, name=f"whtA{c}") for c in range(2)]
        # phase-4 Wh2 blocks, all 8 chunks resident (4.2 KiB/partition)
        w2all = cp.tile([128, NC, IT, W2], BF16)

        # =============== phase 1: Wh = x @ W[h] ===============
        with tc.tile_pool(name="p0", bufs=1) as p0, \
             tc.tile_pool(name="p1s", bufs=4) as p1s, \
             tc.tile_pool(name="p1ps", bufs=1, space="PSUM") as p1ps, \
             tc.tile_pool(name="p1d", bufs=3) as p1d:
            xp1 = p0.tile([128, KT, R], PH1_DT)
            for q in range(8):
                nc.sync.dma_start(xp1[:, q * 4:(q + 1) * 4, :],
                                  x_d[:, q * 4:(q + 1) * 4, :])
            for h in range(NHEADS):
                ps = [[p1ps.tile([128, 512], F32, name=f"ps_{h}_{i}_{oh}",
                                 tag=f"ps{i}{oh}") for oh in range(2)]
                      for i in range(IT)]
                for kb in range(4):
                    wres = p1s.tile([128, 8, NHID], PH1_DT, tag="wres")
                    if h == 0:
                        if kb == 0:
                            nc.scalar.dma_start(
                                wres[:, :2, :], W_d[0, :, 0:2, :])
                            nc.scalar.dma_start(
                                wres[:, 2:, :], W_d[0, :, 2:8, :])
                            # tiny phase-3 vectors behind the critical W chunk
                            nc.scalar.dma_start(
                                a1o_col, a1o_d.rearrange("(b one) -> b one",
                                                         one=1))
                            nc.scalar.dma_start(
                                a2o_b, a2o_d.rearrange(
                                    "(one b) -> one b",
                                    one=1).to_broadcast([128, BIT]))
                        else:
                            nc.scalar.dma_start(
                                wres, W_d[0, :, kb * 8:(kb + 1) * 8, :])
                        if kb == 3:
                            # logit tables + adj behind all four W chunks
                            nc.scalar.dma_start(E1b, E1_d)
                            nc.scalar.dma_start(E1ab, E1a_d)
                            nc.scalar.dma_start(G2, G2_d)
                            nc.scalar.dma_start(G2a, G2a_d)
                            nc.scalar.dma_start(adjT, adj_d)
                    else:
                        nc.sync.dma_start(wres, W_d[h, :, kb * 8:(kb + 1) * 8, :])
                    if PH1_FP8:
                        for kk2 in range(4):
                            k2 = kb * 4 + kk2
                            for i in range(IT):
                                for oh in range(2):
                                    nc.tensor.matmul(
                                        ps[i][oh],
                                        lhsT=xp1[:, 2 * k2:2 * k2 + 2,
                                                 i * 128:(i + 1) * 128],
                                        rhs=wres[:, 2 * kk2:2 * kk2 + 2,
                                                 oh * 512:(oh + 1) * 512],
                                        start=(k2 == 0),
                                        stop=(k2 == KT // 2 - 1),
                                        perf_mode=DR,
                                    )
                    else:
                        for kk in range(8):
                            k = kb * 8 + kk
                            for i in range(IT):
                                for oh in range(2):
                                    nc.tensor.matmul(
                                        ps[i][oh],
                                        lhsT=xp1[:, k, i * 128:(i + 1) * 128],
                                        rhs=wres[:, kk,
                                                 oh * 512:(oh + 1) * 512],
                                        start=(k == 0), stop=(k == KT - 1),
                                    )
                if h == 0:
                    nc.scalar.dma_start(wob, wo_d)
                for i in range(IT):
                    wh_sb = p1d.tile([128, NHID], BF16, tag="wh_sb")
                    nc.vector.tensor_scalar_mul(wh_sb[:, :512], ps[i][0],
                                                INV_SCALE)
                    nc.scalar.mul(wh_sb[:, 512:], ps[i][1], INV_SCALE)
                    # scalar queue keeps sync a pure x/W stream; last head
                    # goes on sync (x/W done) so its dispatches don't block
                    # the ACT engine right when phase 2's first e2 needs it
                    deng = nc.sync if h == NHEADS - 1 else nc.scalar
                    deng.dma_start(
                        ag1_in[h][:, i * NHID:(i + 1) * NHID], wh_sb)
                # allgather this head's Wh while later heads compute
                nc.gpsimd.collective_compute(
                    "AllGather", OP.bypass, ins=[ag1_in[h].opt()],
                    outs=[ag1_out[h].opt()], replica_groups=rg)
                if h == 1:
                    # prefetch head-0's first attention Wh blocks on the Pool
                    # queue: Pool is idle all of phase 1, so its blocking
                    # wait on ag1[0] completion head-of-line-blocks nothing
                    for c in range(2):
                        nc.gpsimd.dma_start(
                            whtA[c], ag1_out[0][c * 128:(c + 1) * 128, :].rearrange(
                                "p (i o) -> p i o", i=IT))

        # =============== phase 2: attention + aggregate, per head ===============
        p2c = ctx.enter_context(tc.tile_pool(name="p2c", bufs=1))
        xcatT = p2c.tile([128, KT, R], F16)

        pps = ctx.enter_context(tc.tile_pool(name="pps", bufs=1, space="PSUM"))
        p2s = ctx.enter_context(tc.tile_pool(name="p2s", bufs=2))
        p2w = ctx.enter_context(tc.tile_pool(name="p2w", bufs=2))
        p2p = ctx.enter_context(tc.tile_pool(name="p2p", bufs=4))
        # phase-3 running sum, fed incrementally as each head's xcat lands
        wh2sb = p2c.tile([BIT, R], F32)

        for h in range(NHEADS):
            rsA = p2s.tile([128, R], F32, tag="rsA")
            nc.gpsimd.memset(rsA, 0.0)

            hps = [pps.tile([128, R], F32, name=f"hps{h}_{os}", tag=f"h{os}")
                   for os in range(8)]
            for c in range(NC):
                if h == 0 and c < 2:
                    wht4 = whtA[c]
                else:
                    wht4 = p2w.tile([128, IT, NHID], BF16, tag="wht", bufs=3)
                    nc.sync.dma_start(
                        wht4, ag1_out[h][c * 128:(c + 1) * 128, :].rearrange(
                            "p (i o) -> p i o", i=IT))
                for i in range(IT):
                    # previous head's elu tails, mid-chunk so the DVE ops
                    # don't delay this chunk's first u tiles; none on the
                    # last chunk, whose final u tiles gate the head's tail
                    if h > 0 and ((c < 6 and i == 2) or
                                  (c == 6 and i in (1, 3))):
                        _elu_tail(c if c < 6 else 6 + (i == 3))
                    jt = c * IT + i
                    e2 = p2p.tile([128, R], BF16, tag="e2")
                    nc.scalar.mul(e2, E1ab[:, h, :], G2a[:, h, i, c:c + 1])
                    m = p2p.tile([128, R], BF16, tag="m")
                    if jt % 2 == 0:
                        nc.vector.scalar_tensor_tensor(
                            out=m, in0=E1b[:, h, :], scalar=G2[:, h, i, c:c + 1],
                            in1=e2, op0=OP.mult, op1=OP.max)
                    else:
                        e1 = p2p.tile([128, R], BF16, tag="e1")
                        nc.scalar.mul(e1, E1b[:, h, :], G2[:, h, i, c:c + 1])
                        nc.vector.tensor_tensor(m, e1, e2, OP.max)
                    u = p2p.tile([128, R], BF16, tag="u")
                    nc.vector.tensor_tensor(u, m, adjT[:, jt, :], OP.mult)
                    nc.gpsimd.tensor_tensor(rsA, rsA, u, OP.add)
                    for os in range(8):
                        nc.tensor.matmul(
                            hps[os], lhsT=wht4[:, i, os * 128:(os + 1) * 128],
                            rhs=u, start=(jt == 0), stop=(jt == JT - 1))

            # eager PSUM drain (banks freed for the next head asap), then the
            # broadcast row-sum + approx-reciprocal normalization chain
            hsb = [p2s.tile([128, R], F32, name=f"hsb{h}_{os}", tag=f"hsb{os}",
                            bufs=1)
                   for os in range(8)]
            # DVE drains in stop order (os 0 stops first in jt=31's os-loop);
            # ACT drains os 7 first so the rowsum matmul (which WARs on
            # hsb[7]) unblocks as early as possible
            for os in (0, 2, 4, 6):
                nc.vector.tensor_copy(hsb[os], hps[os])
            for os in (7, 1, 3, 5):
                nc.scalar.copy(hsb[os], hps[os])
            if h in (1, 2):
                # heads 0/1's phase-3 k-tiles fill the drain windows: PSUM
                # h7 frees first and the PE would otherwise idle on drains.
                # Heads 2-3 stay in the tail so the PE has work to chew
                # while head 3's elu chain runs.
                hp = h - 1
                ph3 = pps.tile([BIT, R], F32, name=f"ph3_{hp}", tag="h7")
                for kk in range(8):
                    nc.tensor.matmul(
                        ph3, lhsT=wob[:, hp * 8 + kk, :],
                        rhs=xcatT[:, hp * 8 + kk, :],
                        start=(kk == 0), stop=(kk == 7))
                if hp == 0:
                    nc.scalar.copy(wh2sb, ph3)
                else:
                    nc.vector.tensor_tensor(wh2sb, wh2sb, ph3, OP.add)
            rsb_ps = pps.tile([128, R], F32, name=f"rsb{h}", tag="h7")
            nc.tensor.matmul(rsb_ps, lhsT=ones128, rhs=rsA, start=True, stop=True)
            rb = p2s.tile([128, R], F32, tag="rb", bufs=1)
            nc.vector.reciprocal(rb, rsb_ps)

            hstage = p2s.tile([128, 8, R], F16, name=f"hstage{h}", tag="hstage",
                              bufs=1)

            def _elu_tail(os, h=h, hstage=hstage):
                mn = p2w.tile([128, R], F16, tag="u2f")
                nc.vector.tensor_scalar_min(mn, hstage[:, os, :], 0.0)
                ex = p2w.tile([128, R], F16, tag="ex")
                nc.scalar.activation(ex, mn, AF.Exp)
                nc.vector.scalar_tensor_tensor(
                    out=xcatT[:, h * 8 + os, :], in0=ex, scalar=-1.0,
                    in1=hstage[:, os, :], op0=OP.add, op1=OP.max)

            for os in range(8):
                eng = nc.gpsimd if os % 3 == 2 else nc.vector
                eng.tensor_tensor(hstage[:, os, :], hsb[os], rb, OP.mult)
                if h == NHEADS - 1:
                    # last head: interleave each mult with its elu tail so
                    # DVE emits xcat k-tiles incrementally — the phase-3
                    # tail matmuls consume them in the same os order
                    _elu_tail(os)

        # =============== phase 3 tail: heads 2-3 + combine ===============
        wh2T_ps = pps.tile([BIT, R], F32, tag="h2")
        for hp in (2, 3):
            for kk in range(8):
                nc.tensor.matmul(
                    wh2T_ps, lhsT=wob[:, hp * 8 + kk, :],
                    rhs=xcatT[:, hp * 8 + kk, :],
                    start=(hp == 2 and kk == 0), stop=(hp == 3 and kk == 7))
        wh2T = p2c.tile([BIT, R], F32)
        nc.vector.tensor_tensor(wh2T, wh2sb, wh2T_ps, OP.add)

        # staged ag2 payload [wh2 (64) | 1.0 | g2] per i-block, two halves:
        # the first collective flies while i-blocks 2-3 are still staging
        agst2 = [p2c.tile([128, 2, W2], BF16, name=f"agst{hf}")
                 for hf in range(2)]
        for hf in range(2):
            nc.vector.memset(agst2[hf][:, :, BIT:BIT + 1], 1.0)
        for i in range(IT):
            hf, sl = divmod(i, 2)
            tp_ps = pps.tile([128, BIT], F32, name=f"w2t{i}", tag="h4")
            nc.tensor.transpose(tp_ps, wh2T[:, i * 128:(i + 1) * 128],
                                ident[:BIT, :BIT])
            nc.vector.tensor_copy(agst2[hf][:, sl, :BIT], tp_ps)
            g2c = p2w.tile([128, 1], F32, tag="g2c")
            scratch2 = p2w.tile([128, BIT], F32, tag="scratch2")
            nc.vector.scalar_tensor_tensor(
                out=scratch2, in0=tp_ps, scalar=0.0, in1=a2o_b,
                op0=OP.bypass, op1=OP.mult, accum_out=g2c)
            nc.vector.tensor_copy(agst2[hf][:, sl, BIT + 1:W2], g2c)
            if sl == 1:
                nc.sync.dma_start(
                    ag2_in2[hf].rearrange("p (i z) -> p i z", i=2), agst2[hf])
                nc.gpsimd.collective_compute(
                    "AllGather", OP.bypass, ins=[ag2_in2[hf].opt()],
                    outs=[ag2_out2[hf].opt()], replica_groups=rg)

        # g1 logit table chain runs during the collectives
        g1T_ps = pps.tile([1, R], F32, tag="h3")
        nc.tensor.matmul(g1T_ps, lhsT=a1o_col, rhs=wh2T, start=True, stop=True)
        g1T = p2c.tile([1, R], F32)
        nc.vector.tensor_copy(g1T, g1T_ps)
        g1b_ps = pps.tile([128, R], F32, tag="h5")
        nc.tensor.matmul(g1b_ps, lhsT=ones_row, rhs=g1T, start=True, stop=True)
        # factored row table exp((1-a)*g1[r]); the exp(a*g1[r]) factor
        # cancels in the output-layer softmax normalization
        E1r = p2c.tile([128, R], BF16)
        nc.scalar.activation(E1r, g1b_ps, AF.Exp, scale=1.0 - ALPHA)

        # =============== phase 4: output attention ===============
        # pull each gathered half into SBUF as its collective lands; exps of
        # the packed g2 columns per half so the i 0-1 chunks start first
        G2o_all = p2c.tile([128, NC, IT, 1], F32)
        G2oa_all = p2c.tile([128, NC, IT, 1], F32)
        for hf in range(2):
            isl = slice(2 * hf, 2 * hf + 2)
            ag2v = ag2_out2[hf].rearrange("(c p) (i z) -> p c i z", c=NC, i=2)
            eng = nc.gpsimd if hf == 0 else nc.sync
            eng.dma_start(w2all[:, :, isl], ag2v)
            nc.scalar.activation(G2o_all[:, :, isl],
                                 w2all[:, :, isl, BIT + 1:W2], AF.Exp)
            nc.scalar.activation(G2oa_all[:, :, isl],
                                 w2all[:, :, isl, BIT + 1:W2], AF.Exp,
                                 scale=ALPHA)
        ht2_ps = pps.tile([BIT + 1, R], F32, tag="h6")
        for ihalf in range(2):
            for c in range(NC):
                for i in (2 * ihalf, 2 * ihalf + 1):
                    jt = c * IT + i
                    m = p2p.tile([128, R], BF16, tag="m")
                    nc.vector.tensor_scalar(
                        out=m, in0=E1r, scalar1=G2o_all[:, c, i, :],
                        scalar2=G2oa_all[:, c, i, :], op0=OP.mult, op1=OP.max)
                    u2 = p2p.tile([128, R], BF16, tag="u")
                    eng = nc.gpsimd if jt % 3 == 0 else nc.vector
                    eng.tensor_tensor(u2, m, adjT[:, jt, :], OP.mult)
                    nc.tensor.matmul(
                        ht2_ps, lhsT=w2all[:, c, i, :BIT + 1], rhs=u2,
                        start=(ihalf == 0 and c == 0 and i == 0),
                        stop=(ihalf == 1 and c == NC - 1 and i == 3))

        # transpose [65, R] (incl the row-sum row), then the row scale is
        # per-partition: one reciprocal + tanh(scale*in) per i-block.
        # separate PSUM tags per i so the four chains pipeline.
        ht2s = p2c.tile([BIT + 1, R], F32)
        ot_tags = ("h1", "h3", "h4", "h5")
        for i in range(IT):
            nc.vector.tensor_copy(ht2s[:, i * 128:(i + 1) * 128],
                                  ht2_ps[:, i * 128:(i + 1) * 128])
            tp_ps = pps.tile([128, BIT + 1], F32, name=f"ot{i}", tag=ot_tags[i])
            nc.tensor.transpose(tp_ps, ht2s[:, i * 128:(i + 1) * 128],
                                ident[:BIT + 1, :BIT + 1])
            rbc = p2w.tile([128, 1], F32, tag="rbc", bufs=2)
            nc.vector.reciprocal(rbc, tp_ps[:, BIT:BIT + 1])
            ob = p2w.tile([128, BIT], F32, tag="ob", bufs=2)
            nc.scalar.activation(ob, tp_ps[:, :BIT], AF.Tanh, scale=rbc)
            nc.sync.dma_start(out_d[i * 128:(i + 1) * 128, :], ob)

    _split_excess_waits(nc, max_waits=1)
    return nc


_CACHED = None


def _get_program():
    global _CACHED
    if _CACHED is None:
        _CACHED = build_program()
    return _CACHED


def _interleave(a, kt):
    """[kt*128, free...] -> [128, kt, free...] partition-major."""
    return np.ascontiguousarray(
        a.reshape(kt, 128, *a.shape[1:]).transpose(1, 0, *range(2, a.ndim + 1)))


def make_in_maps(x, adj, W, a1, a2, W_out, a1_out, a2_out):
    import ml_dtypes
    xT = np.ascontiguousarray(x.T)
    adjT_bf = adj.T.astype(ml_dtypes.bfloat16)
    # host-side tiny logit GEMM: f1/f2 halves for all heads, fp32
    # B = [W[h] @ a1[h] (4 cols) | W[h] @ a2[h] (4 cols)]
    B = np.concatenate(
        [np.stack([W[h] @ a1[h] for h in range(NHEADS)], axis=1),
         np.stack([W[h] @ a2[h] for h in range(NHEADS)], axis=1)],
        axis=1).astype(np.float32)
    F = x.astype(np.float64) @ B.astype(np.float64)   # [N, 8]
    f1 = np.ascontiguousarray(F[:, :NHEADS].T)    # [H, N]
    f2 = np.ascontiguousarray(F[:, NHEADS:].T)    # [H, N]
    E1 = np.exp(f1).astype(ml_dtypes.bfloat16)
    E1a = np.exp(ALPHA * f1).astype(ml_dtypes.bfloat16)
    # f2 exps in [128, H, IT, NC] partition-major j-tile layout
    def _f2sh(a):
        return np.ascontiguousarray(
            a.reshape(NHEADS, NC, IT, 128).transpose(3, 0, 2, 1)
        ).astype(np.float32)
    G2_sh = _f2sh(np.exp(f2 + BIAS_LN))
    G2a_sh = np.ascontiguousarray(
        np.exp(ALPHA * f2 + BIAS_LN).reshape(NHEADS, NC, IT, 128)
        .transpose(3, 0, 2, 1)).astype(np.float32)
    # W interleaved: [h, 128, KT, NHID]; fp8 e4m3 scaled by WS (|v| <= 240
    # keeps e4m3 and e4m3fn bit-identical)
    if PH1_FP8:
        def _q1(a, s):
            return np.clip(a * s, -240.0, 240.0).astype(ml_dtypes.float8_e4m3)
    else:
        def _q1(a, s):
            return a.astype(np.float16)
    W_sh = _q1(np.ascontiguousarray(
        W.reshape(NHEADS, KT, 128, NHID).transpose(0, 2, 1, 3)), WS)
    Wo_sh = _interleave(W_out, KT).astype(np.float16)
    in_maps = []
    for d in range(NC):
        cols = slice(d * R, (d + 1) * R)
        in_maps.append({
            "x_sh": _q1(_interleave(np.ascontiguousarray(xT[:, cols]), KT), XS),
            "W_sh": W_sh,
            "adj_sh": _interleave(np.ascontiguousarray(adjT_bf[:, cols]), JT),
            "Wo_sh": Wo_sh,
            "E1_sh": np.ascontiguousarray(np.broadcast_to(
                E1[None, :, d * R:(d + 1) * R], (128, NHEADS, R))),
            "E1a_sh": np.ascontiguousarray(np.broadcast_to(
                E1a[None, :, d * R:(d + 1) * R], (128, NHEADS, R))),
            "G2_sh": G2_sh, "G2a_sh": G2a_sh,
            "a1_out": a1_out, "a2_out": a2_out,
        })
    return in_maps


def kernel(x, adj, W, a1, a2, W_out, a1_out, a2_out, _trace=False):
    nc = _get_program()
    in_maps = make_in_maps(np.asarray(x, np.float32), np.asarray(adj, np.float32),
                           np.asarray(W, np.float32), np.asarray(a1, np.float32),
                           np.asarray(a2, np.float32), np.asarray(W_out, np.float32),
                           np.asarray(a1_out, np.float32),
                           np.asarray(a2_out, np.float32))
    res = bass_utils.run_bass_kernel_spmd(
        nc, in_maps, core_ids=list(range(NC)), trace=_trace)
    out = np.concatenate([res.results[d]["out_rows"] for d in range(NC)], axis=0)
    if _trace:
        kernel.last_exec_time_ns = res.exec_time_ns
        kernel.last_results = res
    return out

